# revision 8
# baseline (speedup 1.0000x reference)
"""Causal attention head (B=4, S=4096, D=512, E=64) on 8 TRN2 NeuronCores.

Sharding: per batch b, core pair (2b, 2b+1); queries split zig-zag in 16
blocks of 256 so both cores run an identical per-slot key-chunk schedule
jmax_s = 4(s+1), s=0..7 (144 key-chunks/core, ~10% less than the 512-block
padding scheme).  Each core projects the FULL K/V locally (no exchange).

 - K^T/Q^T projected with stationary weights, duplicated into both 64-row
   halves of SBUF for tile_position-packed score matmuls (two 64-contraction
   matmuls fill the 128-row PE).
 - V projected as V^T (stationary weights, long streams) then PE-transposed
   in [64,128] tiles into vp [128kv, 65] with a ones column so the softmax
   denominator falls out of the PV matmul.
 - exp on ScalarE only (36 insts of [128,1024]); all projection PSUM->SBUF
   copies go to DVE/GPSIMD to keep ScalarE free.
 - Causal masking via two [128,1024] bf16 patterns multiplied on the last 4
   chunks of every slot; pattern choice is input data, so the instruction
   stream is identical on all cores (pure SPMD).
 - DMA: all input segments issued up front in the order the interleaved
   (proj seg s -> attention slot s) schedule consumes them.
All matmul inputs bf16 (pre-cast/transposed on host), output f32.
"""

import sys

sys.path.insert(0, "/opt/trn_rl_repo")

import numpy as np
import ml_dtypes

from concourse import bacc, mybir
from concourse import tile
from concourse.bass_utils import run_bass_kernel_spmd

BF16 = ml_dtypes.bfloat16
F32 = mybir.dt.float32
BF = mybir.dt.bfloat16

B, S, D, E = 4, 4096, 512, 64
P = 128
NQ = 2048           # queries per core
QB = 256            # query block (slot) size
NSLOT = NQ // QB    # 8 slots
SEG = 512           # projection segment (columns)
NSEG = S // SEG     # 8 K/V segments
NCH = D // P        # 4 contraction chunks
BLOCKS = {0: [0, 3, 4, 7, 8, 11, 12, 15], 1: [1, 2, 5, 6, 9, 10, 13, 14]}

_CACHE = {}
LAST_RESULT = None


def _build():
    nc = bacc.Bacc(
        "TRN2",
        target_bir_lowering=False,
        debug=False,
        enable_asserts=True,
        num_devices=8,
    )

    xqt_d = nc.declare_dram_parameter("xqt", [D, NQ], BF, isOutput=False)
    xkt_d = nc.declare_dram_parameter("xkt", [D, S], BF, isOutput=False)
    xvt_d = nc.declare_dram_parameter("xvt", [D, S], BF, isOutput=False)
    w_d = nc.declare_dram_parameter("wqkv", [D, 3 * E], BF, isOutput=False)  # wq/8|wk|wv
    masks_d = nc.declare_dram_parameter("masks", [P, 2048], BF, isOutput=False)
    identf_d = nc.declare_dram_parameter("identf", [P, P], F32, isOutput=False)
    zout = nc.declare_dram_parameter("z", [NQ, E], F32, isOutput=True)

    with tile.TileContext(nc) as tc:
        with (
            tc.tile_pool(name="const", bufs=1) as const,
            tc.tile_pool(name="xt", bufs=1) as xt,
            tc.tile_pool(name="proj", bufs=1) as proj,
            tc.tile_pool(name="work", bufs=3) as work,
            tc.tile_pool(name="epi", bufs=2) as epi,
            tc.tile_pool(name="psA", bufs=2, space="PSUM") as psA,
            tc.tile_pool(name="psZ", bufs=2, space="PSUM") as psZ,
            tc.tile_pool(name="psB", bufs=2, space="PSUM") as psB,
        ):
            # ---- constants (issued first on the SP DMA queue) ----
            w_sb = const.tile([P, NCH, 3 * E], BF, tag="w")
            nc.sync.dma_start(
                out=w_sb[:, :, :], in_=w_d.rearrange("(c p) e -> p c e", p=P)
            )
            masks_sb = const.tile([P, 2048], BF, tag="masks")
            nc.sync.dma_start(out=masks_sb[:, :], in_=masks_d[:, :])
            identf_sb = const.tile([P, P], F32, tag="identf")
            nc.sync.dma_start(out=identf_sb[:, :], in_=identf_d[:, :])
            wq_sb = w_sb[:, :, 0:E]
            wk_sb = w_sb[:, :, E : 2 * E]
            wv_sb = w_sb[:, :, 2 * E : 3 * E]

            # ---- persistent SBUF tensors ----
            xqt = xt.tile([P, NCH * NQ], BF, tag="xqt")
            xkt = xt.tile([P, NCH * S], BF, tag="xkt")
            xvt = xt.tile([P, NCH * S], BF, tag="xvt")
            kt2 = proj.tile([P, S], BF, tag="kt2")    # K^T duplicated rows
            qt2 = proj.tile([P, NQ], BF, tag="qt2")   # Q^T duplicated rows
            vts = proj.tile([E, S], F32, tag="vts")   # V^T staging
            vp = proj.tile([P, S // P, E + 1], BF, tag="vp")
            nc.gpsimd.memset(vp[:, :, E : E + 1], 1.0)

            # ---- all input-segment DMAs up front, in consumption order ----
            def dma_seg(dst, src_d, ncols, s):
                nc.sync.dma_start(
                    out=dst[:, :]
                    .rearrange("p (c r) -> p c r", c=NCH)[
                        :, :, s * SEG : (s + 1) * SEG
                    ],
                    in_=src_d[:, s * SEG : (s + 1) * SEG].rearrange(
                        "(c p) r -> p c r", p=P
                    ),
                )

            for s in range(NSEG):
                dma_seg(xkt, xkt_d, S, s)
                if s % 2 == 0:
                    dma_seg(xqt, xqt_d, NQ, s // 2)
                dma_seg(xvt, xvt_d, S, s)

            # ---- per-round building blocks ----
            def proj_dup(w_slice, x, xcols, out2, s):
                """project segment s and write duplicated 64-row halves."""
                ps = psA.tile([E, SEG], F32, tag="st")
                for c in range(NCH):
                    nc.tensor.matmul(
                        ps,
                        lhsT=w_slice[:, c, :],
                        rhs=x[:, c * xcols + s * SEG : c * xcols + (s + 1) * SEG],
                        start=(c == 0),
                        stop=(c == NCH - 1),
                    )
                nc.vector.tensor_copy(out2[0:E, s * SEG : (s + 1) * SEG], ps)
                nc.vector.tensor_copy(out2[E : 2 * E, s * SEG : (s + 1) * SEG], ps)

            def proj_v(s):
                ps = psA.tile([E, SEG], F32, tag="st")
                for c in range(NCH):
                    nc.tensor.matmul(
                        ps,
                        lhsT=wv_sb[:, c, :],
                        rhs=xvt[:, c * S + s * SEG : c * S + (s + 1) * SEG],
                        start=(c == 0),
                        stop=(c == NCH - 1),
                    )
                nc.scalar.copy(vts[:, s * SEG : (s + 1) * SEG], ps)
                tp = psB.tile([P, SEG // P, E], F32, tag="zb")
                for j in range(SEG // P):
                    ch = s * (SEG // P) + j
                    nc.tensor.transpose(
                        tp[:, j, :],
                        vts[:, ch * P : (ch + 1) * P],
                        identf_sb[0:E, 0:E],
                    )
                nc.vector.tensor_copy(
                    vp[:, s * (SEG // P) : (s + 1) * (SEG // P), 0:E], tp
                )

            def attn_slot(s):
                q0 = s * QB
                zps = psZ.tile([E + 1, QB], F32, tag="zt")
                # masked group first so the mask multiply is off the tail
                order = [s] + list(range(s))

                # quarter layout [c0, c2, c1, c3]: each PSUM bank only ever
                # sees one tile_position (bank A: (0,0), bank B: (64,0))
                QMAP = {0: 0, 2: 1, 1: 2, 3: 3}

                def emit_pv(pt, g, first, last):
                    for i in range(4):
                        nc.tensor.matmul(
                            zps,
                            lhsT=vp[:, 4 * g + i, :],
                            rhs=pt[:, QMAP[i] * QB : (QMAP[i] + 1) * QB],
                            start=(first and i == 0),
                            stop=(last and i == 3),
                            skip_group_check=True,
                        )

                prev = None
                for n, g in enumerate(order):
                    sps = psA.tile([P, 4 * QB], F32, tag="st")
                    for i in range(4):
                        j = 4 * g + i
                        h2 = i % 2
                        nc.tensor.matmul(
                            sps[:, QMAP[i] * QB : (QMAP[i] + 1) * QB],
                            lhsT=kt2[h2 * E : (h2 + 1) * E, j * P : (j + 1) * P],
                            rhs=qt2[h2 * E : (h2 + 1) * E, q0 : q0 + QB],
                            start=True,
                            stop=True,
                            tile_position=(h2 * E, 0),
                        )
                    pt = work.tile([P, 4 * QB], BF, tag="pt")
                    nc.scalar.activation(
                        out=pt, in_=sps, func=mybir.ActivationFunctionType.Exp
                    )
                    if g == s:
                        nc.vector.tensor_mul(
                            pt, pt, masks_sb[:, (s % 2) * 1024 : (s % 2 + 1) * 1024]
                        )
                    if prev is not None:
                        emit_pv(*prev, first=(n == 1), last=False)
                    prev = (pt, g)
                emit_pv(*prev, first=(s == 0), last=True)

                # epilogue: Z^T/denom -> Z rows -> DRAM
                zsb = epi.tile([E + 1, QB], F32, tag="zsb")
                nc.vector.tensor_copy(zsb, zps)
                for u in range(QB // P):
                    zbp = psB.tile([P, E + 1], F32, tag="zb")
                    nc.tensor.transpose(
                        zbp,
                        zsb[:, u * P : (u + 1) * P],
                        identf_sb[0 : E + 1, 0 : E + 1],
                    )
                    rc = epi.tile([P, 1], F32, tag="rc")
                    nc.vector.reciprocal(rc, zbp[:, E : E + 1])
                    zf = epi.tile([P, E], F32, tag="zf")
                    nc.vector.tensor_scalar_mul(zf, zbp[:, 0:E], rc)
                    row0 = q0 + u * P
                    nc.sync.dma_start(out=zout[row0 : row0 + P, :], in_=zf)

            # ---- interleaved schedule ----
            for s in range(NSEG):
                proj_dup(wk_sb, xkt, S, kt2, s)
                if s % 2 == 0:
                    proj_dup(wq_sb, xqt, NQ, qt2, s // 2)
                proj_v(s)
                attn_slot(s)

    nc.compile()
    return nc


def _get_nc():
    if "nc" not in _CACHE:
        _CACHE["nc"] = _build()
    return _CACHE["nc"]


def _ensure_ntff_hook():
    """Install antenv.axon_hooks + NTFF profile hook if the image lacks it."""
    import types

    try:
        from antenv import axon_hooks  # noqa: F401

        return
    except ImportError:
        pass
    import antenv
    from concourse import bass_utils as _bu

    mod = types.ModuleType("antenv.axon_hooks")
    _state = {}
    mod.set_axon_ntff_profile_hook = lambda h: _state.__setitem__("h", h)
    mod.get_axon_ntff_profile_hook = lambda: _state.get("h")
    sys.modules["antenv.axon_hooks"] = mod
    antenv.axon_hooks = mod
    sys.path.insert(0, "/root/.axon_site/trn_agent_boot")
    from trn_boot import _ntff_profile_via_ctypes

    mod.set_axon_ntff_profile_hook(
        _ntff_profile_via_ctypes("/opt/axon/libaxon_pjrt.so")
    )
    _bu.upload_artifacts = lambda tmpdir: f"local://{tmpdir}"


def _mask_patterns():
    # quarter order [c0, c2, c1, c3] to match the kernel's bank layout
    kl = np.arange(P)[:, None]
    ql = np.arange(QB)[None, :]
    diag0 = (kl <= ql).astype(np.float32)
    diag1 = (kl <= ql - P).astype(np.float32)
    ones = np.ones((P, QB), np.float32)
    zero = np.zeros((P, QB), np.float32)
    x = np.concatenate([diag0, zero, diag1, zero], axis=1)
    y = np.concatenate([ones, diag0, ones, diag1], axis=1)
    return x.astype(BF16), y.astype(BF16)


def kernel(key_inputs, value_inputs, query_inputs, Wq, Wk, Wv):
    global LAST_RESULT
    import os

    key_inputs = np.asarray(key_inputs, dtype=np.float32)
    value_inputs = np.asarray(value_inputs, dtype=np.float32)
    query_inputs = np.asarray(query_inputs, dtype=np.float32)
    wqkv = np.concatenate(
        [
            np.asarray(Wq, dtype=np.float32) * 0.125,
            np.asarray(Wk, dtype=np.float32),
            np.asarray(Wv, dtype=np.float32),
        ],
        axis=1,
    ).astype(BF16)
    mx, my = _mask_patterns()
    masks_np = {
        0: np.concatenate([mx, my], axis=1),  # even slots X, odd Y
        1: np.concatenate([my, mx], axis=1),  # even slots Y, odd X
    }
    identf_np = np.eye(P, dtype=np.float32)

    in_maps = []
    for c in range(8):
        b, h = c // 2, c % 2
        xq_c = np.concatenate(
            [query_inputs[b, 256 * blk : 256 * blk + QB] for blk in BLOCKS[h]],
            axis=0,
        )
        in_maps.append(
            {
                "xqt": np.ascontiguousarray(xq_c.T).astype(BF16),
                "xkt": np.ascontiguousarray(key_inputs[b].T).astype(BF16),
                "xvt": np.ascontiguousarray(value_inputs[b].T).astype(BF16),
                "wqkv": wqkv,
                "masks": masks_np[h],
                "identf": identf_np,
            }
        )

    nc = _get_nc()
    trace = bool(int(os.environ.get("KERNEL_TRACE", "0")))
    if trace:
        _ensure_ntff_hook()
    res = run_bass_kernel_spmd(
        nc,
        in_maps,
        core_ids=list(range(8)),
        trace=trace,
        tmpdir=os.environ.get("KERNEL_TRACE_DIR") or None,
    )
    LAST_RESULT = res

    out = np.empty((B, S, E), dtype=np.float32)
    for c in range(8):
        b, h = c // 2, c % 2
        z = np.asarray(res.results[c]["z"], dtype=np.float32)
        for s, blk in enumerate(BLOCKS[h]):
            out[b, 256 * blk : 256 * blk + QB] = z[s * QB : (s + 1) * QB]
    return out
